# revision 7
# baseline (speedup 1.0000x reference)
"""Llama GQA attention layer (prefill with KV cache) as a Trainium2 Bass/Tile
kernel, tensor-parallel over heads across 8 NeuronCores.

Contract: kernel(**inputs) takes the FULL unsharded inputs (numpy, fp32) and
returns the FULL [B, S, H] output. Sharding: each core gets 4 q-heads and the
matching kv-head (w_qkv column shard, w_o row shard); hidden_states is
replicated (fed pre-transposed); the o_proj row-parallel all-reduce is a host
numpy sum over the 8 partial outputs.

Self-contained: hardcodes all shapes; only imports the toolchain from
/opt/trn_rl_repo.
"""

import sys

if "/opt/trn_rl_repo" not in sys.path:
    sys.path.insert(0, "/opt/trn_rl_repo")

import numpy as np

import concourse.bass as bass
import concourse.mybir as mybir
import concourse.tile as tile
from concourse import bacc
from concourse.bass_utils import run_bass_kernel_spmd
from concourse.masks import make_identity

# Problem shapes
B, S, P = 2, 1024, 1024
T = P + S                      # 2048 total kv positions
H, NQ, NKV, D = 4096, 32, 8, 128
G = NQ // NKV                  # 4 q heads per kv head
NCORES = 8
GPC = NQ // NCORES             # 4 q heads per core
SCALE = 1.0 / float(np.sqrt(D))

BS = B * S                     # 2048 tokens (b-major)
QKV_COLS = GPC * D + 2 * D     # 768 per-core qkv output columns
KCH = 32                       # H // 128 contraction chunks
MCH = QKV_COLS // 128          # 6 output chunks (0-3 q, 4 k, 5 v)
NB = BS // 512                 # 4 token blocks in phase 1
F32 = mybir.dt.float32
F32R = mybir.dt.float32r


def _r(ap):
    """Bitcast an fp32 AP to float32r for full-rate PE matmuls."""
    return ap.bitcast(F32R)


def _build_program():
    nc = bacc.Bacc("TRN2", target_bir_lowering=False, debug=False,
                   num_devices=NCORES)

    xT = nc.dram_tensor("xT", [H, BS], F32R, kind="ExternalInput").ap()
    wqkv = nc.dram_tensor("wqkv", [128, KCH * QKV_COLS], F32R,
                          kind="ExternalInput").ap()
    wo = nc.dram_tensor("wo", [128, GPC * H], F32R, kind="ExternalInput").ap()
    cosT_d = nc.dram_tensor("cosT", [128, S], F32R, kind="ExternalInput").ap()
    ssinT_d = nc.dram_tensor("ssinT", [128, S], F32R, kind="ExternalInput").ap()
    kcT_d = nc.dram_tensor("kcT", [128, B * P], F32R, kind="ExternalInput").ap()
    vc_d = nc.dram_tensor("vc", [B * P, D], F32R, kind="ExternalInput").ap()
    masks_d = nc.dram_tensor("masks", [128, 4 * 512], F32R,
                             kind="ExternalInput").ap()
    y = nc.dram_tensor("y", [BS, H], F32, kind="ExternalOutput").ap()

    with tile.TileContext(nc) as tc:
        with tc.tile_pool(name="persist", bufs=1) as pp:
            # Layouts (all [128 partitions, free]):
            #  qT: head-dim on partitions, cols g*2048 + b*1024 + s
            #  kT: cols b*2048 + t  (t<1024 cache, t>=1024 new)
            #  v_sb: [t, d] chunks; chunk (b, tc) at col 128*(16b+tc),
            #        tc 0-7 cache, 8-15 new
            qT = pp.tile([128, GPC * BS], F32R, tag="qT")
            kT = pp.tile([128, B * T], F32R, tag="kT")
            v_sb = pp.tile([128, B * T], F32R, tag="v_sb")
            vt_stage = pp.tile([128, BS], F32, tag="vt_stage")
            cosT = pp.tile([128, S], F32R, tag="cosT")
            ssinT = pp.tile([128, S], F32R, tag="ssinT")
            masks_sb = pp.tile([128, 4 * 512], F32R, tag="masks")
            ident = pp.tile([128, 128], F32, tag="ident")
            ones = pp.tile([128, 1], F32, tag="ones")

            nc.sync.dma_start(cosT[:], cosT_d[:])
            nc.sync.dma_start(ssinT[:], ssinT_d[:])
            nc.sync.dma_start(masks_sb[:], masks_d[:])
            # KV cache loads straight into their attention-time slots.
            for b in range(B):
                nc.sync.dma_start(kT[:, b * T:b * T + P],
                                  kcT_d[:, b * P:(b + 1) * P])
                for tch in range(P // 128):
                    nc.sync.dma_start(
                        v_sb[:, 128 * (16 * b + tch):128 * (16 * b + tch + 1)],
                        vc_d[b * P + 128 * tch:b * P + 128 * (tch + 1), :])
            nc.vector.memset(ones[:], 1.0)
            make_identity(nc, ident[:])

            # ---- Phase 1: fused QKV projection (transposed outputs) ----
            with (tc.tile_pool(name="wq", bufs=1) as wqp,
                  tc.tile_pool(name="xt", bufs=8) as xtp,
                  tc.tile_pool(name="ps1", bufs=6, space="PSUM") as ps1):
                wq_sb = wqp.tile([128, KCH * QKV_COLS], F32R, tag="wq_sb")
                for kk in range(8):  # 4 contraction chunks per group
                    nc.sync.dma_start(
                        wq_sb[:, kk * 4 * QKV_COLS:(kk + 1) * 4 * QKV_COLS],
                        wqkv[:, kk * 4 * QKV_COLS:(kk + 1) * 4 * QKV_COLS])

                for nb in range(NB):
                    psums = [ps1.tile([128, 512], F32, tag="qkvps", name=f"qkvps{nb}_{m}")
                             for m in range(MCH)]
                    for kk in range(8):
                        xts = []
                        for k4 in range(4):
                            k = 4 * kk + k4
                            xt_t = xtp.tile([128, 512], F32R, tag="xt", name=f"xt{nb}_{k}")
                            nc.sync.dma_start(
                                xt_t[:], xT[128 * k:128 * (k + 1),
                                            512 * nb:512 * (nb + 1)])
                            xts.append(xt_t)
                        for m in range(MCH):
                            for k4 in range(4):
                                k = 4 * kk + k4
                                nc.tensor.matmul(
                                    psums[m][:],
                                    wq_sb[:, k * QKV_COLS + 128 * m:
                                             k * QKV_COLS + 128 * (m + 1)],
                                    xts[k4][:],
                                    start=(k == 0), stop=(k == KCH - 1))
                    # psum -> transposed-layout SBUF destinations
                    b = nb // 2
                    s0 = (nb % 2) * 512
                    for m in range(MCH):
                        if m < GPC:
                            dst = qT[:, m * BS + nb * 512:m * BS + nb * 512 + 512]
                        elif m == GPC:
                            dst = kT[:, b * T + P + s0:b * T + P + s0 + 512]
                        else:
                            dst = vt_stage[:, nb * 512:nb * 512 + 512]
                        nc.scalar.copy(dst, psums[m][:])

            # ---- Phase 2/3: RoPE, attention, o_proj ----
            with (tc.tile_pool(name="wop", bufs=1) as wop,
                  tc.tile_pool(name="otp", bufs=1) as otp,
                  tc.tile_pool(name="probs", bufs=3) as probsp,
                  tc.tile_pool(name="rope", bufs=3) as ropep,
                  tc.tile_pool(name="recip", bufs=1) as recipp,
                  tc.tile_pool(name="bcast", bufs=1) as bcastp,
                  tc.tile_pool(name="yp", bufs=2) as yp,
                  tc.tile_pool(name="ps_sc", bufs=2, space="PSUM") as ps_sc,
                  tc.tile_pool(name="ps_ot", bufs=2, space="PSUM") as ps_ot,
                  tc.tile_pool(name="ps_sum", bufs=2, space="PSUM") as ps_sum,
                  tc.tile_pool(name="ps_o", bufs=2, space="PSUM") as ps_o):
                wo_sb = wop.tile([128, GPC * H], F32R, tag="wo_sb")
                outT_sb = otp.tile([128, B * GPC * S], F32R, tag="outT_sb")
                for gg in range(4):
                    nc.sync.dma_start(wo_sb[:, gg * H:(gg + 1) * H],
                                      wo[:, gg * H:(gg + 1) * H])

                # New-V transpose into [t, d] layout
                for b in range(B):
                    for tch in range(S // 128):
                        ps = ps_sc.tile([128, 128], F32, tag="sc")
                        nc.tensor.transpose(
                            ps[:],
                            vt_stage[:, b * S + 128 * tch:b * S + 128 * (tch + 1)],
                            ident[:])
                        nc.vector.tensor_copy(
                            v_sb[:, 128 * (16 * b + 8 + tch):
                                 128 * (16 * b + 9 + tch)], ps[:])

                # RoPE in place on q heads and new k
                def rope_inplace(src, col0):
                    for cc in range(2):
                        c0 = col0 + 512 * cc
                        sl = slice(512 * cc, 512 * (cc + 1))
                        rot = ropep.tile([128, 512], F32R, tag="rt",
                                         name="rot")
                        nc.sync.dma_start(rot[0:64, :],
                                          src[64:128, c0:c0 + 512])
                        nc.sync.dma_start(rot[64:128, :],
                                          src[0:64, c0:c0 + 512])
                        nc.vector.tensor_mul(rot[:], rot[:], ssinT[:, sl])
                        t2 = ropep.tile([128, 512], F32R, tag="rt", name="rt2")
                        nc.vector.tensor_mul(t2[:], src[:, c0:c0 + 512],
                                             cosT[:, sl])
                        nc.vector.tensor_add(src[:, c0:c0 + 512],
                                             rot[:], t2[:])

                for b in range(B):
                    for g in range(GPC):
                        rope_inplace(qT, g * BS + b * S)
                    rope_inplace(kT, b * T + P)

                # Attention + o_proj
                for b in range(B):
                    for g in range(GPC):
                        qcol = g * BS + b * S
                        for j in range(2):  # 512-wide s blocks
                            scol = qcol + j * 512
                            n_t = (P // 128) + 4 * (j + 1)  # causal skip
                            sums_ps = ps_sum.tile([1, 512], F32, tag="sums")
                            outT_ps = ps_ot.tile([128, 512], F32, tag="ot")
                            for ti in range(n_t):
                                if ti < 8:
                                    kcol = b * T + 128 * ti
                                    vch = 16 * b + ti
                                else:
                                    kcol = b * T + P + 128 * (ti - 8)
                                    vch = 16 * b + ti
                                sc_ps = ps_sc.tile([128, 512], F32, tag="sc")
                                nc.tensor.matmul(
                                    sc_ps[:], kT[:, kcol:kcol + 128],
                                    qT[:, scol:scol + 512],
                                    start=True, stop=True)
                                pt = probsp.tile([128, 512], F32R, tag="pt")
                                nc.scalar.activation(
                                    pt[:], sc_ps[:],
                                    mybir.ActivationFunctionType.Exp,
                                    scale=SCALE)
                                if ti >= 8:
                                    r_idx = (ti - 8) - 4 * j
                                    if 0 <= r_idx < 4:
                                        nc.vector.tensor_mul(
                                            pt[:], pt[:],
                                            masks_sb[:, 512 * r_idx:
                                                     512 * (r_idx + 1)])
                                first, last = ti == 0, ti == n_t - 1
                                nc.tensor.matmul(sums_ps[:], _r(ones[:]),
                                                 pt[:],
                                                 start=first, stop=last)
                                nc.tensor.matmul(
                                    outT_ps[:],
                                    v_sb[:, 128 * vch:128 * (vch + 1)],
                                    pt[:], start=first, stop=last)
                            rc = recipp.tile([1, 512], F32, tag="rc")
                            nc.vector.reciprocal(rc[0:1, :], sums_ps[0:1, :])
                            bc = bcastp.tile([128, 512], F32, tag="bc")
                            nc.gpsimd.partition_broadcast(bc[:], rc[0:1, :])
                            ocol = b * GPC * S + g * S + j * 512
                            nc.vector.tensor_mul(
                                outT_sb[:, ocol:ocol + 512], outT_ps[:], bc[:])

                    # o_proj for this batch (overlaps next batch's attention)
                    for sc in range(S // 128):
                        for hb in range(H // 512):
                            ops = ps_o.tile([128, 512], F32, tag="op")
                            for g in range(GPC):
                                lcol = b * GPC * S + g * S + 128 * sc
                                nc.tensor.matmul(
                                    ops[:],
                                    outT_sb[:, lcol:lcol + 128],
                                    wo_sb[:, g * H + 512 * hb:
                                             g * H + 512 * (hb + 1)],
                                    start=(g == 0), stop=(g == GPC - 1))
                            ys = yp.tile([128, 512], F32, tag="ys")
                            nc.vector.tensor_copy(ys[:], ops[:])
                            nc.sync.dma_start(
                                y[b * S + 128 * sc:b * S + 128 * (sc + 1),
                                  512 * hb:512 * (hb + 1)], ys[:])

    nc.compile()
    return nc


_PROGRAM = None


def _get_program():
    global _PROGRAM
    if _PROGRAM is None:
        _PROGRAM = _build_program()
    return _PROGRAM


def _shard_inputs(hidden_states, w_qkv, w_o, cos, sin, k_cache, v_cache):
    """Build the 8 per-core input maps (numpy, fp32)."""
    hs = np.ascontiguousarray(np.asarray(hidden_states, np.float32))
    w_qkv = np.asarray(w_qkv, np.float32)
    w_o = np.asarray(w_o, np.float32)
    cos = np.asarray(cos, np.float32)
    sin = np.asarray(sin, np.float32)
    k_cache = np.asarray(k_cache, np.float32)
    v_cache = np.asarray(v_cache, np.float32)

    xT = np.ascontiguousarray(hs.reshape(BS, H).T)
    cosT = np.ascontiguousarray(cos.T)
    ssinT = np.ascontiguousarray(sin.T)
    ssinT[0:64] *= -1.0

    # 4 multiplicative causal mask tiles: mask_r[t, s] = (s - t >= 128*r)
    tl = np.arange(128)[:, None]
    sl = np.arange(512)[None, :]
    masks = np.concatenate(
        [(sl - tl >= 128 * r).astype(np.float32) for r in range(4)], axis=1)
    masks = np.ascontiguousarray(masks)

    in_maps = []
    for c in range(NCORES):
        wq_c = w_qkv[:, c * GPC * D:(c + 1) * GPC * D]
        wk_c = w_qkv[:, NQ * D + c * D:NQ * D + (c + 1) * D]
        wv_c = w_qkv[:, (NQ + NKV) * D + c * D:(NQ + NKV) * D + (c + 1) * D]
        wc = np.concatenate([wq_c, wk_c, wv_c], axis=1)      # [H, 768]
        wqkv_r = np.ascontiguousarray(
            wc.reshape(KCH, 128, QKV_COLS).transpose(1, 0, 2)
            .reshape(128, KCH * QKV_COLS))
        wo_c = w_o[c * GPC * D:(c + 1) * GPC * D, :]          # [512, H]
        wo_r = np.ascontiguousarray(
            wo_c.reshape(GPC, 128, H).transpose(1, 0, 2).reshape(128, GPC * H))
        kcT = np.ascontiguousarray(
            k_cache[:, :, c, :].reshape(B * P, D).T)          # [128, 2048]
        vc = np.ascontiguousarray(v_cache[:, :, c, :].reshape(B * P, D))
        in_maps.append(dict(xT=xT, wqkv=wqkv_r, wo=wo_r, cosT=cosT,
                            ssinT=ssinT, kcT=kcT, vc=vc, masks=masks))
    return in_maps


def _run(in_maps, trace=False):
    nc = _get_program()
    return run_bass_kernel_spmd(nc, in_maps, list(range(NCORES)), trace=trace)


def kernel(hidden_states, w_qkv, w_o, cos, sin, k_cache, v_cache):
    in_maps = _shard_inputs(hidden_states, w_qkv, w_o, cos, sin,
                            k_cache, v_cache)
    res = _run(in_maps)
    acc = np.zeros((BS, H), np.float64)
    for c in range(NCORES):
        acc += res.results[c]["y"]
    return acc.astype(np.float32).reshape(B, S, H)


# revision 9
# speedup vs baseline: 1.0477x; 1.0477x over previous
"""Llama GQA attention layer (prefill with KV cache) as a Trainium2 Bass/Tile
kernel, tensor-parallel over heads across 8 NeuronCores.

Contract: kernel(**inputs) takes the FULL unsharded inputs (numpy, fp32) and
returns the FULL [B, S, H] output. Sharding: each core gets 4 q-heads and the
matching kv-head (w_qkv column shard, w_o row shard); hidden_states is
replicated (fed pre-transposed); the o_proj row-parallel all-reduce is a host
numpy sum over the 8 partial outputs.

Self-contained: hardcodes all shapes; only imports the toolchain from
/opt/trn_rl_repo.
"""

import sys

if "/opt/trn_rl_repo" not in sys.path:
    sys.path.insert(0, "/opt/trn_rl_repo")

import numpy as np

import concourse.bass as bass
import concourse.mybir as mybir
import concourse.tile as tile
from concourse import bacc
from concourse.bass_utils import run_bass_kernel_spmd
from concourse.masks import make_identity

# Problem shapes
B, S, P = 2, 1024, 1024
T = P + S                      # 2048 total kv positions
H, NQ, NKV, D = 4096, 32, 8, 128
G = NQ // NKV                  # 4 q heads per kv head
NCORES = 8
GPC = NQ // NCORES             # 4 q heads per core
SCALE = 1.0 / float(np.sqrt(D))

BS = B * S                     # 2048 tokens (b-major)
QKV_COLS = GPC * D + 2 * D     # 768 per-core qkv output columns
KCH = 32                       # H // 128 contraction chunks
MCH = QKV_COLS // 128          # 6 output chunks (0-3 q, 4 k, 5 v)
NB = BS // 512                 # 4 token blocks in phase 1
F32 = mybir.dt.float32
F32R = mybir.dt.float32r


def _r(ap):
    """Bitcast an fp32 AP to float32r for full-rate PE matmuls."""
    return ap.bitcast(F32R)


def _build_program():
    nc = bacc.Bacc("TRN2", target_bir_lowering=False, debug=False,
                   num_devices=NCORES)

    xT = nc.dram_tensor("xT", [H, BS], F32R, kind="ExternalInput").ap()
    wqkv = nc.dram_tensor("wqkv", [128, KCH * QKV_COLS], F32R,
                          kind="ExternalInput").ap()
    wo = nc.dram_tensor("wo", [128, GPC * H], F32R, kind="ExternalInput").ap()
    cosT_d = nc.dram_tensor("cosT", [128, S], F32R, kind="ExternalInput").ap()
    ssinT_d = nc.dram_tensor("ssinT", [128, S], F32R, kind="ExternalInput").ap()
    kcT_d = nc.dram_tensor("kcT", [128, B * P], F32R, kind="ExternalInput").ap()
    vc_d = nc.dram_tensor("vc", [B * P, D], F32R, kind="ExternalInput").ap()
    masks_d = nc.dram_tensor("masks", [128, 4 * 512], F32R,
                             kind="ExternalInput").ap()
    y = nc.dram_tensor("y", [BS, H], F32, kind="ExternalOutput").ap()

    with tile.TileContext(nc) as tc:
        with tc.tile_pool(name="persist", bufs=1) as pp:
            # Layouts (all [128 partitions, free]):
            #  qT: head-dim on partitions, cols g*2048 + b*1024 + s
            #  kT: cols b*2048 + t  (t<1024 cache, t>=1024 new)
            #  v_sb: [t, d] chunks; chunk (b, tc) at col 128*(16b+tc),
            #        tc 0-7 cache, 8-15 new
            qT = pp.tile([128, GPC * BS], F32R, tag="qT")
            kT = pp.tile([128, B * T], F32R, tag="kT")
            v_sb = pp.tile([128, B * T], F32R, tag="v_sb")
            vt_stage = pp.tile([128, BS], F32, tag="vt_stage")
            cosT = pp.tile([128, S], F32R, tag="cosT")
            ssinT = pp.tile([128, S], F32R, tag="ssinT")
            masks_sb = pp.tile([128, 4 * 512], F32R, tag="masks")
            ident = pp.tile([128, 128], F32, tag="ident")
            ones = pp.tile([128, 1], F32, tag="ones")
            ones_r = pp.tile([1, 128], F32, tag="ones_r")

            nc.sync.dma_start(cosT[:], cosT_d[:])
            nc.sync.dma_start(ssinT[:], ssinT_d[:])
            nc.sync.dma_start(masks_sb[:], masks_d[:])
            # KV cache loads straight into their attention-time slots.
            for b in range(B):
                nc.sync.dma_start(kT[:, b * T:b * T + P],
                                  kcT_d[:, b * P:(b + 1) * P])
                for tch in range(P // 128):
                    nc.sync.dma_start(
                        v_sb[:, 128 * (16 * b + tch):128 * (16 * b + tch + 1)],
                        vc_d[b * P + 128 * tch:b * P + 128 * (tch + 1), :])
            nc.vector.memset(ones[:], 1.0)
            nc.vector.memset(ones_r[:], 1.0)
            make_identity(nc, ident[:])

            # ---- Phase 1: fused QKV projection (transposed outputs) ----
            with (tc.tile_pool(name="wq", bufs=1) as wqp,
                  tc.tile_pool(name="xt", bufs=8) as xtp,
                  tc.tile_pool(name="rope", bufs=3) as ropep,
                  tc.tile_pool(name="ps1", bufs=6, space="PSUM") as ps1,
                  tc.tile_pool(name="ps_tr", bufs=1, space="PSUM") as ps_tr):

                def rope_chunk(src_ap, c0, s0):
                    rot = ropep.tile([128, 512], F32R, tag="rt", name="rot")
                    nc.sync.dma_start(rot[0:64, :],
                                      src_ap[64:128, c0:c0 + 512])
                    nc.sync.dma_start(rot[64:128, :],
                                      src_ap[0:64, c0:c0 + 512])
                    nc.vector.tensor_mul(rot[:], rot[:],
                                         ssinT[:, s0:s0 + 512])
                    t2 = ropep.tile([128, 512], F32R, tag="rt", name="rt2")
                    nc.vector.tensor_mul(t2[:], src_ap[:, c0:c0 + 512],
                                         cosT[:, s0:s0 + 512])
                    nc.vector.tensor_add(src_ap[:, c0:c0 + 512],
                                         rot[:], t2[:])
                wq_sb = wqp.tile([128, KCH * QKV_COLS], F32R, tag="wq_sb")
                for kk in range(8):  # 4 contraction chunks per group
                    nc.sync.dma_start(
                        wq_sb[:, kk * 4 * QKV_COLS:(kk + 1) * 4 * QKV_COLS],
                        wqkv[:, kk * 4 * QKV_COLS:(kk + 1) * 4 * QKV_COLS])

                for nb in range(NB):
                    psums = [ps1.tile([128, 512], F32, tag="qkvps", name=f"qkvps{nb}_{m}")
                             for m in range(MCH)]
                    for kk in range(8):
                        xts = []
                        for k4 in range(4):
                            k = 4 * kk + k4
                            xt_t = xtp.tile([128, 512], F32R, tag="xt", name=f"xt{nb}_{k}")
                            nc.sync.dma_start(
                                xt_t[:], xT[128 * k:128 * (k + 1),
                                            512 * nb:512 * (nb + 1)])
                            xts.append(xt_t)
                        for m in range(MCH):
                            for k4 in range(4):
                                k = 4 * kk + k4
                                nc.tensor.matmul(
                                    psums[m][:],
                                    wq_sb[:, k * QKV_COLS + 128 * m:
                                             k * QKV_COLS + 128 * (m + 1)],
                                    xts[k4][:],
                                    start=(k == 0), stop=(k == KCH - 1))
                    # psum -> transposed-layout SBUF destinations
                    b = nb // 2
                    s0 = (nb % 2) * 512
                    for m in range(MCH):
                        if m < GPC:
                            dst = qT[:, m * BS + nb * 512:m * BS + nb * 512 + 512]
                        elif m == GPC:
                            dst = kT[:, b * T + P + s0:b * T + P + s0 + 512]
                        else:
                            dst = vt_stage[:, nb * 512:nb * 512 + 512]
                        nc.scalar.copy(dst, psums[m][:])
                    for g in range(GPC):
                        rope_chunk(qT, g * BS + b * S + s0, s0)
                    rope_chunk(kT, b * T + P + s0, s0)
                    for i in range(4):
                        tok0 = nb * 512 + 128 * i
                        ps_t = ps_tr.tile([128, 128], F32, tag="tr",
                                          name=f"tr{nb}_{i}")
                        nc.tensor.transpose(ps_t[:],
                                            vt_stage[:, tok0:tok0 + 128],
                                            ident[:])
                        vch_new = 16 * b + 8 + s0 // 128 + i
                        nc.vector.tensor_copy(
                            v_sb[:, 128 * vch_new:128 * (vch_new + 1)],
                            ps_t[:])

            # ---- Phase 2/3: RoPE, attention, o_proj ----
            with (tc.tile_pool(name="wop", bufs=1) as wop,
                  tc.tile_pool(name="otp", bufs=1) as otp,
                  tc.tile_pool(name="probs", bufs=4) as probsp,
                  tc.tile_pool(name="recip", bufs=1) as recipp,
                  tc.tile_pool(name="bcast", bufs=1) as bcastp,
                  tc.tile_pool(name="yp", bufs=2) as yp,
                  tc.tile_pool(name="ps_sc", bufs=3, space="PSUM") as ps_sc,
                  tc.tile_pool(name="ps_ot", bufs=2, space="PSUM") as ps_ot,
                  tc.tile_pool(name="ps_sum", bufs=1, space="PSUM") as ps_sum,
                  tc.tile_pool(name="ps_o", bufs=2, space="PSUM") as ps_o):
                wo_sb = wop.tile([128, GPC * H], F32R, tag="wo_sb")
                outT_sb = otp.tile([128, B * GPC * S], F32R, tag="outT_sb")
                for gg in range(4):
                    nc.sync.dma_start(wo_sb[:, gg * H:(gg + 1) * H],
                                      wo[:, gg * H:(gg + 1) * H])

                # Attention + o_proj
                for b in range(B):
                    for g in range(GPC):
                        qcol = g * BS + b * S
                        for j in range(2):  # 512-wide s blocks
                            scol = qcol + j * 512
                            n_t = (P // 128) + 4 * (j + 1)  # causal skip
                            sums_ps = ps_sum.tile([1, 512], F32, tag="sums")
                            outT_ps = ps_ot.tile([128, 512], F32, tag="ot")
                            for ti in range(n_t):
                                if ti < 8:
                                    kcol = b * T + 128 * ti
                                    vch = 16 * b + ti
                                else:
                                    kcol = b * T + P + 128 * (ti - 8)
                                    vch = 16 * b + ti
                                sc_ps = ps_sc.tile([128, 512], F32, tag="sc")
                                nc.tensor.matmul(
                                    sc_ps[:], kT[:, kcol:kcol + 128],
                                    qT[:, scol:scol + 512],
                                    start=True, stop=True)
                                pt = probsp.tile([128, 512], F32R, tag="pt")
                                nc.scalar.activation(
                                    pt[:], sc_ps[:],
                                    mybir.ActivationFunctionType.Exp,
                                    scale=SCALE)
                                if ti >= 8:
                                    r_idx = (ti - 8) - 4 * j
                                    if 0 <= r_idx < 4:
                                        nc.vector.tensor_mul(
                                            pt[:], pt[:],
                                            masks_sb[:, 512 * r_idx:
                                                     512 * (r_idx + 1)])
                                first, last = ti == 0, ti == n_t - 1
                                nc.tensor.matmul(sums_ps[:], _r(ones[:]),
                                                 pt[:],
                                                 start=first, stop=last)
                                nc.tensor.matmul(
                                    outT_ps[:],
                                    v_sb[:, 128 * vch:128 * (vch + 1)],
                                    pt[:], start=first, stop=last)
                            rc = recipp.tile([1, 512], F32R, tag="rc")
                            with nc.allow_low_precision(
                                    reason="f32r recip for softmax norm"):
                                nc.vector.reciprocal(rc[0:1, :],
                                                     sums_ps[0:1, :])
                            bc_ps = ps_sc.tile([128, 512], F32, tag="sc",
                                               name="bc_ps")
                            nc.tensor.matmul(bc_ps[:], _r(ones_r[:]),
                                             rc[0:1, :],
                                             start=True, stop=True)
                            bc = bcastp.tile([128, 512], F32, tag="bc")
                            nc.vector.tensor_copy(bc[:], bc_ps[:])
                            ocol = b * GPC * S + g * S + j * 512
                            nc.vector.tensor_mul(
                                outT_sb[:, ocol:ocol + 512], outT_ps[:], bc[:])

                    # o_proj for this batch (overlaps next batch's attention)
                    for sc in range(S // 128):
                        for hb in range(H // 512):
                            ops = ps_o.tile([128, 512], F32, tag="op")
                            for g in range(GPC):
                                lcol = b * GPC * S + g * S + 128 * sc
                                nc.tensor.matmul(
                                    ops[:],
                                    outT_sb[:, lcol:lcol + 128],
                                    wo_sb[:, g * H + 512 * hb:
                                             g * H + 512 * (hb + 1)],
                                    start=(g == 0), stop=(g == GPC - 1))
                            ys = yp.tile([128, 512], F32, tag="ys")
                            nc.vector.tensor_copy(ys[:], ops[:])
                            nc.sync.dma_start(
                                y[b * S + 128 * sc:b * S + 128 * (sc + 1),
                                  512 * hb:512 * (hb + 1)], ys[:])

    nc.compile()
    return nc


_PROGRAM = None


def _get_program():
    global _PROGRAM
    if _PROGRAM is None:
        _PROGRAM = _build_program()
    return _PROGRAM


def _shard_inputs(hidden_states, w_qkv, w_o, cos, sin, k_cache, v_cache):
    """Build the 8 per-core input maps (numpy, fp32)."""
    hs = np.ascontiguousarray(np.asarray(hidden_states, np.float32))
    w_qkv = np.asarray(w_qkv, np.float32)
    w_o = np.asarray(w_o, np.float32)
    cos = np.asarray(cos, np.float32)
    sin = np.asarray(sin, np.float32)
    k_cache = np.asarray(k_cache, np.float32)
    v_cache = np.asarray(v_cache, np.float32)

    xT = np.ascontiguousarray(hs.reshape(BS, H).T)
    cosT = np.ascontiguousarray(cos.T)
    ssinT = np.ascontiguousarray(sin.T)
    ssinT[0:64] *= -1.0

    # 4 multiplicative causal mask tiles: mask_r[t, s] = (s - t >= 128*r)
    tl = np.arange(128)[:, None]
    sl = np.arange(512)[None, :]
    masks = np.concatenate(
        [(sl - tl >= 128 * r).astype(np.float32) for r in range(4)], axis=1)
    masks = np.ascontiguousarray(masks)

    in_maps = []
    for c in range(NCORES):
        wq_c = w_qkv[:, c * GPC * D:(c + 1) * GPC * D]
        wk_c = w_qkv[:, NQ * D + c * D:NQ * D + (c + 1) * D]
        wv_c = w_qkv[:, (NQ + NKV) * D + c * D:(NQ + NKV) * D + (c + 1) * D]
        wc = np.concatenate([wq_c, wk_c, wv_c], axis=1)      # [H, 768]
        wqkv_r = np.ascontiguousarray(
            wc.reshape(KCH, 128, QKV_COLS).transpose(1, 0, 2)
            .reshape(128, KCH * QKV_COLS))
        wo_c = w_o[c * GPC * D:(c + 1) * GPC * D, :]          # [512, H]
        wo_r = np.ascontiguousarray(
            wo_c.reshape(GPC, 128, H).transpose(1, 0, 2).reshape(128, GPC * H))
        kcT = np.ascontiguousarray(
            k_cache[:, :, c, :].reshape(B * P, D).T)          # [128, 2048]
        vc = np.ascontiguousarray(v_cache[:, :, c, :].reshape(B * P, D))
        in_maps.append(dict(xT=xT, wqkv=wqkv_r, wo=wo_r, cosT=cosT,
                            ssinT=ssinT, kcT=kcT, vc=vc, masks=masks))
    return in_maps


def _run(in_maps, trace=False):
    nc = _get_program()
    return run_bass_kernel_spmd(nc, in_maps, list(range(NCORES)), trace=trace)


def kernel(hidden_states, w_qkv, w_o, cos, sin, k_cache, v_cache):
    in_maps = _shard_inputs(hidden_states, w_qkv, w_o, cos, sin,
                            k_cache, v_cache)
    res = _run(in_maps)
    acc = np.zeros((BS, H), np.float64)
    for c in range(NCORES):
        acc += res.results[c]["y"]
    return acc.astype(np.float32).reshape(B, S, H)


# revision 10
# speedup vs baseline: 1.1240x; 1.0728x over previous
"""Llama GQA attention layer (prefill with KV cache) as a Trainium2 Bass/Tile
kernel, tensor-parallel over heads across 8 NeuronCores.

Contract: kernel(**inputs) takes the FULL unsharded inputs (numpy, fp32) and
returns the FULL [B, S, H] output. Sharding: each core gets 4 q-heads and the
matching kv-head (w_qkv column shard, w_o row shard); hidden_states is
replicated (fed pre-transposed); the o_proj row-parallel all-reduce is a host
numpy sum over the 8 partial outputs.

Self-contained: hardcodes all shapes; only imports the toolchain from
/opt/trn_rl_repo.
"""

import sys

if "/opt/trn_rl_repo" not in sys.path:
    sys.path.insert(0, "/opt/trn_rl_repo")

import numpy as np

import concourse.bass as bass
import concourse.mybir as mybir
import concourse.tile as tile
from concourse import bacc
from concourse.bass_utils import run_bass_kernel_spmd
from concourse.masks import make_identity

# Problem shapes
B, S, P = 2, 1024, 1024
T = P + S                      # 2048 total kv positions
H, NQ, NKV, D = 4096, 32, 8, 128
G = NQ // NKV                  # 4 q heads per kv head
NCORES = 8
GPC = NQ // NCORES             # 4 q heads per core
SCALE = 1.0 / float(np.sqrt(D))

BS = B * S                     # 2048 tokens (b-major)
QKV_COLS = GPC * D + 2 * D     # 768 per-core qkv output columns
KCH = 32                       # H // 128 contraction chunks
MCH = QKV_COLS // 128          # 6 output chunks (0-3 q, 4 k, 5 v)
NB = BS // 512                 # 4 token blocks in phase 1
F32 = mybir.dt.float32
F32R = mybir.dt.float32r


def _r(ap):
    """Bitcast an fp32 AP to float32r for full-rate PE matmuls."""
    return ap.bitcast(F32R)


def _build_program():
    nc = bacc.Bacc("TRN2", target_bir_lowering=False, debug=False,
                   num_devices=NCORES)

    xT = nc.dram_tensor("xT", [H, BS], F32R, kind="ExternalInput").ap()
    wqkv = nc.dram_tensor("wqkv", [128, KCH * QKV_COLS], F32R,
                          kind="ExternalInput").ap()
    wo = nc.dram_tensor("wo", [128, GPC * H], F32R, kind="ExternalInput").ap()
    cosT_d = nc.dram_tensor("cosT", [128, S], F32R, kind="ExternalInput").ap()
    ssinT_d = nc.dram_tensor("ssinT", [128, S], F32R, kind="ExternalInput").ap()
    kcT_d = nc.dram_tensor("kcT", [128, B * P], F32R, kind="ExternalInput").ap()
    vc_d = nc.dram_tensor("vc", [B * P, D], F32R, kind="ExternalInput").ap()
    masks_d = nc.dram_tensor("masks", [128, 4 * 512], F32R,
                             kind="ExternalInput").ap()
    y = nc.dram_tensor("y", [BS, H], F32, kind="ExternalOutput").ap()

    with tile.TileContext(nc) as tc:
        with tc.tile_pool(name="persist", bufs=1) as pp:
            # Layouts (all [128 partitions, free]):
            #  qT: head-dim on partitions, cols g*2048 + b*1024 + s
            #  kT: cols b*2048 + t  (t<1024 cache, t>=1024 new)
            #  v_sb: [t, d] chunks; chunk (b, tc) at col 128*(16b+tc),
            #        tc 0-7 cache, 8-15 new
            qT = pp.tile([128, GPC * BS], F32R, tag="qT")
            kT = pp.tile([128, B * T], F32R, tag="kT")
            v_sb = pp.tile([128, B * T], F32R, tag="v_sb")
            vt_stage = pp.tile([128, BS], F32, tag="vt_stage")
            cosT = pp.tile([128, S], F32R, tag="cosT")
            ssinT = pp.tile([128, S], F32R, tag="ssinT")
            masks_sb = pp.tile([128, 4 * 512], F32R, tag="masks")
            ident = pp.tile([128, 128], F32, tag="ident")
            ones = pp.tile([128, 1], F32, tag="ones")
            ones_r = pp.tile([1, 128], F32, tag="ones_r")

            nc.sync.dma_start(cosT[:], cosT_d[:])
            nc.sync.dma_start(ssinT[:], ssinT_d[:])
            nc.sync.dma_start(masks_sb[:], masks_d[:])
            # KV cache loads straight into their attention-time slots.
            for b in range(B):
                nc.sync.dma_start(kT[:, b * T:b * T + P],
                                  kcT_d[:, b * P:(b + 1) * P])
                for tch in range(P // 128):
                    nc.sync.dma_start(
                        v_sb[:, 128 * (16 * b + tch):128 * (16 * b + tch + 1)],
                        vc_d[b * P + 128 * tch:b * P + 128 * (tch + 1), :])
            nc.vector.memset(ones[:], 1.0)
            nc.vector.memset(ones_r[:], 1.0)
            make_identity(nc, ident[:])

            # ---- Phase 1: fused QKV projection (transposed outputs) ----
            with (tc.tile_pool(name="wq", bufs=1) as wqp,
                  tc.tile_pool(name="xt", bufs=8) as xtp,
                  tc.tile_pool(name="rope", bufs=3) as ropep,
                  tc.tile_pool(name="ps1", bufs=6, space="PSUM") as ps1,
                  tc.tile_pool(name="ps_tr", bufs=1, space="PSUM") as ps_tr):

                def rope_chunk(src_ap, c0, s0):
                    rot = ropep.tile([128, 512], F32R, tag="rt", name="rot")
                    nc.sync.dma_start(rot[0:64, :],
                                      src_ap[64:128, c0:c0 + 512])
                    nc.sync.dma_start(rot[64:128, :],
                                      src_ap[0:64, c0:c0 + 512])
                    nc.vector.tensor_mul(rot[:], rot[:],
                                         ssinT[:, s0:s0 + 512])
                    t2 = ropep.tile([128, 512], F32R, tag="rt", name="rt2")
                    nc.vector.tensor_mul(t2[:], src_ap[:, c0:c0 + 512],
                                         cosT[:, s0:s0 + 512])
                    nc.vector.tensor_add(src_ap[:, c0:c0 + 512],
                                         rot[:], t2[:])
                wq_sb = wqp.tile([128, KCH * QKV_COLS], F32R, tag="wq_sb")
                for kk in range(8):  # 4 contraction chunks per group
                    nc.sync.dma_start(
                        wq_sb[:, kk * 4 * QKV_COLS:(kk + 1) * 4 * QKV_COLS],
                        wqkv[:, kk * 4 * QKV_COLS:(kk + 1) * 4 * QKV_COLS])

                for nb in range(NB):
                    psums = [ps1.tile([128, 512], F32, tag="qkvps", name=f"qkvps{nb}_{m}")
                             for m in range(MCH)]
                    for kk in range(8):
                        xts = []
                        for k4 in range(4):
                            k = 4 * kk + k4
                            xt_t = xtp.tile([128, 512], F32R, tag="xt", name=f"xt{nb}_{k}")
                            nc.sync.dma_start(
                                xt_t[:], xT[128 * k:128 * (k + 1),
                                            512 * nb:512 * (nb + 1)])
                            xts.append(xt_t)
                        for m in range(MCH):
                            for k4 in range(4):
                                k = 4 * kk + k4
                                nc.tensor.matmul(
                                    psums[m][:],
                                    wq_sb[:, k * QKV_COLS + 128 * m:
                                             k * QKV_COLS + 128 * (m + 1)],
                                    xts[k4][:],
                                    start=(k == 0), stop=(k == KCH - 1))
                    # psum -> transposed-layout SBUF destinations
                    b = nb // 2
                    s0 = (nb % 2) * 512
                    for m in range(MCH):
                        if m < GPC:
                            dst = qT[:, m * BS + nb * 512:m * BS + nb * 512 + 512]
                        elif m == GPC:
                            dst = kT[:, b * T + P + s0:b * T + P + s0 + 512]
                        else:
                            dst = vt_stage[:, nb * 512:nb * 512 + 512]
                        nc.scalar.copy(dst, psums[m][:])
                    for g in range(GPC):
                        rope_chunk(qT, g * BS + b * S + s0, s0)
                    rope_chunk(kT, b * T + P + s0, s0)
                    for i in range(4):
                        tok0 = nb * 512 + 128 * i
                        ps_t = ps_tr.tile([128, 128], F32, tag="tr",
                                          name=f"tr{nb}_{i}")
                        nc.tensor.transpose(ps_t[:],
                                            vt_stage[:, tok0:tok0 + 128],
                                            ident[:])
                        vch_new = 16 * b + 8 + s0 // 128 + i
                        nc.vector.tensor_copy(
                            v_sb[:, 128 * vch_new:128 * (vch_new + 1)],
                            ps_t[:])

            # ---- Phase 2/3: RoPE, attention, o_proj ----
            with (tc.tile_pool(name="wop", bufs=1) as wop,
                  tc.tile_pool(name="otp", bufs=1) as otp,
                  tc.tile_pool(name="probs", bufs=4) as probsp,
                  tc.tile_pool(name="recip", bufs=1) as recipp,
                  tc.tile_pool(name="bcast", bufs=1) as bcastp,
                  tc.tile_pool(name="yp", bufs=2) as yp,
                  tc.tile_pool(name="ps_sc", bufs=2, space="PSUM") as ps_sc,
                  tc.tile_pool(name="ps_ot", bufs=2, space="PSUM") as ps_ot,
                  tc.tile_pool(name="ps_sum", bufs=2, space="PSUM") as ps_sum,
                  tc.tile_pool(name="ps_o", bufs=2, space="PSUM") as ps_o):
                wo_sb = wop.tile([128, GPC * H], F32R, tag="wo_sb")
                outT_sb = otp.tile([128, B * GPC * S], F32R, tag="outT_sb")
                for gg in range(4):
                    nc.sync.dma_start(wo_sb[:, gg * H:(gg + 1) * H],
                                      wo[:, gg * H:(gg + 1) * H])

                # Attention + o_proj (finalize deferred one s-block to
                # keep the recip/broadcast chain off the PE critical path)
                def finalize(f):
                    f_sums, f_outT, f_ocol = f
                    rc = recipp.tile([1, 512], F32R, tag="rc", name="rc")
                    with nc.allow_low_precision(
                            reason="f32r recip for softmax norm"):
                        nc.vector.reciprocal(rc[0:1, :], f_sums[0:1, :])
                    bc_ps = ps_sc.tile([128, 512], F32, tag="sc",
                                       name="bc_ps")
                    nc.tensor.matmul(bc_ps[:], _r(ones_r[:]), rc[0:1, :],
                                     start=True, stop=True)
                    bc = bcastp.tile([128, 512], F32, tag="bc", name="bc")
                    nc.vector.tensor_copy(bc[:], bc_ps[:])
                    nc.vector.tensor_mul(
                        outT_sb[:, f_ocol:f_ocol + 512], f_outT[:], bc[:])

                pending = None
                for b in range(B):
                    for g in range(GPC):
                        qcol = g * BS + b * S
                        for j in range(2):  # 512-wide s blocks
                            scol = qcol + j * 512
                            n_t = (P // 128) + 4 * (j + 1)  # causal skip
                            sums_ps = ps_sum.tile([1, 512], F32, tag="sums")
                            outT_ps = ps_ot.tile([128, 512], F32, tag="ot")
                            for ti in range(n_t):
                                if ti < 8:
                                    kcol = b * T + 128 * ti
                                    vch = 16 * b + ti
                                else:
                                    kcol = b * T + P + 128 * (ti - 8)
                                    vch = 16 * b + ti
                                sc_ps = ps_sc.tile([128, 512], F32, tag="sc")
                                nc.tensor.matmul(
                                    sc_ps[:], kT[:, kcol:kcol + 128],
                                    qT[:, scol:scol + 512],
                                    start=True, stop=True)
                                pt = probsp.tile([128, 512], F32R, tag="pt")
                                nc.scalar.activation(
                                    pt[:], sc_ps[:],
                                    mybir.ActivationFunctionType.Exp,
                                    scale=SCALE)
                                if ti >= 8:
                                    r_idx = (ti - 8) - 4 * j
                                    if 0 <= r_idx < 4:
                                        nc.vector.tensor_mul(
                                            pt[:], pt[:],
                                            masks_sb[:, 512 * r_idx:
                                                     512 * (r_idx + 1)])
                                first, last = ti == 0, ti == n_t - 1
                                nc.tensor.matmul(sums_ps[:], _r(ones[:]),
                                                 pt[:],
                                                 start=first, stop=last)
                                nc.tensor.matmul(
                                    outT_ps[:],
                                    v_sb[:, 128 * vch:128 * (vch + 1)],
                                    pt[:], start=first, stop=last)
                            ocol = b * GPC * S + g * S + j * 512
                            if pending is not None:
                                finalize(pending)
                            pending = (sums_ps, outT_ps, ocol)

                    # o_proj for this batch (overlaps next batch's attention)
                    if pending is not None:
                        finalize(pending)
                        pending = None
                    for sc in range(S // 128):
                        for hb in range(H // 512):
                            ops = ps_o.tile([128, 512], F32, tag="op")
                            for g in range(GPC):
                                lcol = b * GPC * S + g * S + 128 * sc
                                nc.tensor.matmul(
                                    ops[:],
                                    outT_sb[:, lcol:lcol + 128],
                                    wo_sb[:, g * H + 512 * hb:
                                             g * H + 512 * (hb + 1)],
                                    start=(g == 0), stop=(g == GPC - 1))
                            ys = yp.tile([128, 512], F32, tag="ys")
                            if hb % 2 == 0:
                                nc.vector.tensor_copy(ys[:], ops[:])
                            else:
                                nc.scalar.copy(ys[:], ops[:])
                            nc.sync.dma_start(
                                y[b * S + 128 * sc:b * S + 128 * (sc + 1),
                                  512 * hb:512 * (hb + 1)], ys[:])

    nc.compile()
    return nc


_PROGRAM = None


def _get_program():
    global _PROGRAM
    if _PROGRAM is None:
        _PROGRAM = _build_program()
    return _PROGRAM


def _shard_inputs(hidden_states, w_qkv, w_o, cos, sin, k_cache, v_cache):
    """Build the 8 per-core input maps (numpy, fp32)."""
    hs = np.ascontiguousarray(np.asarray(hidden_states, np.float32))
    w_qkv = np.asarray(w_qkv, np.float32)
    w_o = np.asarray(w_o, np.float32)
    cos = np.asarray(cos, np.float32)
    sin = np.asarray(sin, np.float32)
    k_cache = np.asarray(k_cache, np.float32)
    v_cache = np.asarray(v_cache, np.float32)

    xT = np.ascontiguousarray(hs.reshape(BS, H).T)
    cosT = np.ascontiguousarray(cos.T)
    ssinT = np.ascontiguousarray(sin.T)
    ssinT[0:64] *= -1.0

    # 4 multiplicative causal mask tiles: mask_r[t, s] = (s - t >= 128*r)
    tl = np.arange(128)[:, None]
    sl = np.arange(512)[None, :]
    masks = np.concatenate(
        [(sl - tl >= 128 * r).astype(np.float32) for r in range(4)], axis=1)
    masks = np.ascontiguousarray(masks)

    in_maps = []
    for c in range(NCORES):
        wq_c = w_qkv[:, c * GPC * D:(c + 1) * GPC * D]
        wk_c = w_qkv[:, NQ * D + c * D:NQ * D + (c + 1) * D]
        wv_c = w_qkv[:, (NQ + NKV) * D + c * D:(NQ + NKV) * D + (c + 1) * D]
        wc = np.concatenate([wq_c, wk_c, wv_c], axis=1)      # [H, 768]
        wqkv_r = np.ascontiguousarray(
            wc.reshape(KCH, 128, QKV_COLS).transpose(1, 0, 2)
            .reshape(128, KCH * QKV_COLS))
        wo_c = w_o[c * GPC * D:(c + 1) * GPC * D, :]          # [512, H]
        wo_r = np.ascontiguousarray(
            wo_c.reshape(GPC, 128, H).transpose(1, 0, 2).reshape(128, GPC * H))
        kcT = np.ascontiguousarray(
            k_cache[:, :, c, :].reshape(B * P, D).T)          # [128, 2048]
        vc = np.ascontiguousarray(v_cache[:, :, c, :].reshape(B * P, D))
        in_maps.append(dict(xT=xT, wqkv=wqkv_r, wo=wo_r, cosT=cosT,
                            ssinT=ssinT, kcT=kcT, vc=vc, masks=masks))
    return in_maps


def _run(in_maps, trace=False):
    nc = _get_program()
    return run_bass_kernel_spmd(nc, in_maps, list(range(NCORES)), trace=trace)


def kernel(hidden_states, w_qkv, w_o, cos, sin, k_cache, v_cache):
    in_maps = _shard_inputs(hidden_states, w_qkv, w_o, cos, sin,
                            k_cache, v_cache)
    res = _run(in_maps)
    acc = np.zeros((BS, H), np.float64)
    for c in range(NCORES):
        acc += res.results[c]["y"]
    return acc.astype(np.float32).reshape(B, S, H)
